# revision 21
# baseline (speedup 1.0000x reference)
"""CoSTCo model kernel for 8x Trainium2 NeuronCores.

Math: out[b] = relu(wfc2 @ relu(wfc1 @ h2[b] + bfc1) + bfc2), where
  h2[b] = relu(Q02[i0[b]*64 + i2[b]] + Q1[i1[b]])
  Q_m   = relu(emb_m @ w1.T + b1) @ w2[:, :, m].T        (weight folding)
  Q02[i*64+j] = Q0[i] + Q2[j] + b2                       (pair fusion)

conv1 (over rank) and conv2 (over modes) act linearly on each gathered
embedding row, so they fold into per-table lookup matrices Q_m computed
once on the host (tables are tiny: 339/5825/64 rows). Modes 0 and 2 fuse
further into one 21696-row pair table, so the device does 2 dma_gathers
per batch element instead of 3. Gathers alternate across the 4 SWDGE
queues so descriptor generation spreads over the GPSIMD core pairs.

Device per 512-batch block: 1 DVE add, 8 PE transposes into [channel,
batch] layout, relu(+bias) drains, and the MLP (256->256->1) on the
tensor engine.

Sharding: pure data parallel over the batch dim, 16384 elements per core.
"""

import sys
import types

sys.path.insert(0, "/opt/trn_rl_repo")

import ml_dtypes
import numpy as np

# ---------------------------------------------------------------- constants
B = 131072
N_CORES = 8
BPC = B // N_CORES          # 16384 batch elements per core
CHUNK = 1024                # idx per dma_gather instruction (Q7 scratch caps this)
NCHUNK = BPC // CHUNK
RANK = 128
C = 256                     # channels
FIELD_DIMS = (339, 5825, 64)
F02 = FIELD_DIMS[0] * FIELD_DIMS[2]   # fused pair-table rows
NSWQ = 4                    # SWDGE queues in use

TDT = "bf16"                # gather-table dtype: "f32" | "bf16"
MDT = "bf16"                # matmul/activation dtype: "f32" | "f32r" | "bf16"


def _install_ntff_hook():
    """antenv in this image lacks axon_hooks; inject it and register the
    ctypes NTFF profiling hook so trace=True works under axon."""
    import antenv

    if "antenv.axon_hooks" in sys.modules:
        return
    mod = types.ModuleType("antenv.axon_hooks")
    mod._hook = None
    mod.set_axon_ntff_profile_hook = lambda h: setattr(mod, "_hook", h)
    mod.get_axon_ntff_profile_hook = lambda: mod._hook
    sys.modules["antenv.axon_hooks"] = mod
    antenv.axon_hooks = mod
    try:
        from trn_agent_boot.trn_boot import _ntff_profile_via_ctypes

        mod._hook = _ntff_profile_via_ctypes("/opt/axon/libaxon_pjrt.so")
    except Exception:
        pass


_NC_CACHE = {}


def _build(bpc=BPC, chunk=CHUNK, tdt=None, mdt=None):
    """Build + compile the per-core Bass program. Identical on all cores;
    per-core data arrives via in_maps."""
    import concourse.bass as bass
    import concourse.tile as tile
    from concourse import bacc, mybir

    tdt = tdt or TDT
    mdt = mdt or MDT
    key = (bpc, chunk, tdt, mdt)
    if key in _NC_CACHE:
        return _NC_CACHE[key]

    f32 = mybir.dt.float32
    i16 = mybir.dt.int16
    Alu = mybir.AluOpType
    Act = mybir.ActivationFunctionType
    nchunk = bpc // chunk
    nblk = chunk // 512
    ngrp = chunk // 128

    tdt_ = {"f32": f32, "bf16": mybir.dt.bfloat16}[tdt]
    # storage dtype of s/h2/h3/weights, and the matmul-operand view dtype
    sdt = mybir.dt.bfloat16 if mdt == "bf16" else f32
    mmdt = {"f32": f32, "f32r": mybir.dt.float32r,
            "bf16": mybir.dt.bfloat16}[mdt]

    def mm_view(ap):
        return ap.bitcast(mmdt) if mmdt != sdt else ap

    nc = bacc.Bacc("TRN2", target_bir_lowering=False, debug=False,
                   num_devices=N_CORES, num_swdge_queues=NSWQ,
                   dynamic_dma_scratch_size=65536)

    # DRAM inputs (per-core shards / replicated folded weights)
    q02_dram = nc.dram_tensor("q02", [F02, C], tdt_, kind="ExternalInput")
    q1_dram = nc.dram_tensor("q1", [FIELD_DIMS[1], C], tdt_,
                             kind="ExternalInput")
    idx_dram = nc.dram_tensor("idxw", [2, 128, bpc // 16], i16,
                              kind="ExternalInput")
    w1t_dram = nc.dram_tensor("w1t", [2, 128, C], sdt, kind="ExternalInput")
    w2t_dram = nc.dram_tensor("w2t", [128, 2], sdt, kind="ExternalInput")
    b1_dram = nc.dram_tensor("b1t", [128, 2], f32, kind="ExternalInput")
    b3_dram = nc.dram_tensor("b3t", [1, 1], f32, kind="ExternalInput")
    id_dram = nc.dram_tensor("ident", [128, 128], sdt, kind="ExternalInput")
    out_dram = nc.dram_tensor("out", [bpc], f32, kind="ExternalOutput")
    out_view = out_dram.ap().rearrange("(c n) -> c n", n=chunk)

    with tile.TileContext(nc) as tc:
        with (
            tc.tile_pool(name="const", bufs=1) as const_pool,
            tc.tile_pool(name="gat", bufs=4) as gat_pool,
            tc.tile_pool(name="sum", bufs=3) as sum_pool,
            tc.tile_pool(name="act", bufs=2) as act_pool,
            tc.tile_pool(name="stage", bufs=3) as stage_pool,
            tc.tile_pool(name="pt", bufs=2, space="PSUM") as pt_pool,
            tc.tile_pool(name="ph", bufs=3, space="PSUM") as ph_pool,
            tc.tile_pool(name="po", bufs=1, space="PSUM") as po_pool,
        ):
            # --- constants into SBUF
            ident = const_pool.tile([128, 128], sdt)
            nc.sync.dma_start(ident[:], id_dram.ap())
            w1t = []                                  # fc1 weights, c-chunked
            for j in range(2):
                wt = const_pool.tile([128, C], sdt, tag=f"w1t{j}")
                nc.sync.dma_start(wt[:], w1t_dram.ap()[j])
                w1t.append(wt)
            w2t = const_pool.tile([128, 2], sdt)      # fc2 weights, c-chunked
            nc.sync.dma_start(w2t[:], w2t_dram.ap())
            b1s = const_pool.tile([128, 2], f32)
            nc.sync.dma_start(b1s[:], b1_dram.ap())
            b3s = const_pool.tile([1, 1], f32)
            nc.sync.dma_start(b3s[:], b3_dram.ap())
            idxs = []
            for m in range(2):
                it = const_pool.tile([128, bpc // 16], i16, tag=f"idx{m}")
                nc.sync.dma_start(it[:], idx_dram.ap()[m])
                idxs.append(it)

            cw = chunk // 16  # idx columns per chunk
            for ch in range(nchunk):
                # --- gather table rows for this chunk: [128, ngrp, 256]
                g = []
                for m, src in enumerate((q02_dram, q1_dram)):
                    dst = gat_pool.tile([128, ngrp, C], tdt_, tag=f"g{m}")
                    nc.gpsimd.dma_gather(
                        dst[:], src.ap(),
                        idxs[m][:, ch * cw:(ch + 1) * cw],
                        chunk, chunk, C,
                        queue_num=(2 * ch + m) % NSWQ,
                    )
                    g.append(dst)

                stage = stage_pool.tile([1, chunk], f32)
                for blk in range(nblk):
                    gs = slice(4 * blk, 4 * blk + 4)
                    # --- s = g02 + g1  (row layout [batch, channel])
                    s = sum_pool.tile([128, 4, C], sdt)
                    nc.vector.tensor_tensor(s[:], g[0][:, gs, :],
                                            g[1][:, gs, :], Alu.add)
                    # --- transpose to [channel, batch], 2 halves of 128
                    ps = pt_pool.tile([128, 1024], sdt, tag="pt")
                    for h in range(2):
                        for grp in range(4):
                            nc.tensor.transpose(
                                ps[:, h * 512 + grp * 128:
                                   h * 512 + (grp + 1) * 128],
                                s[:, grp, h * 128:(h + 1) * 128],
                                ident[:],
                            )
                    # --- h2 = relu(sum)  (b2 folded into q02 on host)
                    h2t = act_pool.tile([128, 1024], sdt, tag="h2")
                    nc.scalar.activation(h2t[:], ps[:], Act.Relu)
                    h2 = [h2t[:, 0:512], h2t[:, 512:1024]]
                    # --- fc1: h3 = relu(wfc1 @ h2 + bfc1), 2 output halves
                    h3 = []
                    for h in range(2):
                        ph = ph_pool.tile([128, 512], f32, tag="ph")
                        for j in range(2):
                            nc.tensor.matmul(
                                ph[:],
                                mm_view(w1t[j][:, h * 128:(h + 1) * 128]),
                                mm_view(h2[j]),
                                start=(j == 0), stop=(j == 1),
                            )
                        hs = act_pool.tile([128, 512], sdt, tag=f"h3{h}")
                        if h == 0:
                            nc.vector.tensor_scalar(hs[:], ph[:],
                                                    b1s[:, h:h + 1], 0.0,
                                                    Alu.add, Alu.max)
                        else:
                            nc.scalar.activation(hs[:], ph[:], Act.Relu,
                                                 bias=b1s[:, h:h + 1])
                        h3.append(hs)
                    # --- fc2: out = relu(wfc2 @ h3 + bfc2)
                    po = po_pool.tile([128, 512], f32, tag="po")
                    for j in range(2):
                        nc.tensor.matmul(po[0:1, :],
                                         mm_view(w2t[:, j:j + 1]),
                                         mm_view(h3[j][:]),
                                         start=(j == 0), stop=(j == 1))
                    nc.scalar.activation(
                        stage[0:1, blk * 512:(blk + 1) * 512], po[0:1, :],
                        Act.Relu, bias=b3s[0:1, 0:1])
                nc.sync.dma_start(out_view[ch:ch + 1, :], stage[:])

    nc.compile()
    _NC_CACHE[key] = nc
    return nc


def _fold_tables(inputs):
    """Q_m = relu(emb_m @ w1.T + b1) @ w2[:,:,m].T in float64, then the
    mode-0/2 pair fusion Q02[i*64+j] = Q0[i] + Q2[j] + b2."""
    w1_ = np.asarray(inputs["w1"]).astype(np.float64)
    b1_ = np.asarray(inputs["b1"]).astype(np.float64)
    w2 = np.asarray(inputs["w2"])
    qs = []
    for m, emb in enumerate((inputs["emb0"], inputs["emb1"], inputs["emb2"])):
        r = np.maximum(np.asarray(emb).astype(np.float64) @ w1_.T + b1_, 0.0)
        qs.append(r @ w2[:, :, m].astype(np.float64).T)
    q02 = (qs[0][:, None, :] + qs[2][None, :, :]
           + np.asarray(inputs["b2"]).astype(np.float64)).reshape(F02, C)
    return q02, qs[1]


def _np_dt(name):
    return {"f32": np.float32, "bf16": ml_dtypes.bfloat16}[name]


def _make_common(inputs, tdt=None, mdt=None):
    tdt = tdt or TDT
    mdt = mdt or MDT
    q02, q1 = _fold_tables(inputs)
    tnp = _np_dt(tdt)
    snp = _np_dt("bf16" if mdt == "bf16" else "f32")
    return {
        "q02": np.ascontiguousarray(q02.astype(tnp)),
        "q1": np.ascontiguousarray(q1.astype(tnp)),
        "w1t": np.ascontiguousarray(
            np.asarray(inputs["wfc1"]).T.astype(snp).reshape(2, 128, C)),
        "w2t": np.ascontiguousarray(
            np.asarray(inputs["wfc2"]).reshape(C).astype(snp)
            .reshape(2, 128).T),
        "b1t": np.ascontiguousarray(
            np.asarray(inputs["bfc1"]).astype(np.float32).reshape(2, 128).T),
        "b3t": np.asarray(inputs["bfc2"]).astype(np.float32).reshape(1, 1),
        "ident": np.eye(128, dtype=snp),
    }


def _wrap_idx(idx, chunk):
    """Wrap a 1-D int array into dma_gather's [128, n/16] int16 layout,
    chunk by chunk: logical position k of chunk c lives at
    [k % 16, c*chunk/16 + k // 16], replicated across the 8 Q7 cores."""
    n = idx.shape[0]
    w = (idx.reshape(n // chunk, chunk // 16, 16)
         .transpose(0, 2, 1).reshape(n // chunk, 16, chunk // 16))
    wrapped = np.concatenate(list(w), axis=1).astype(np.int16)  # [16, n/16]
    return np.tile(wrapped, (8, 1))                             # [128, n/16]


def _make_idxw(shard, chunk=CHUNK):
    """shard: [n, 3] int indices -> ([2, 128, n/16] int16 wrapped layout,
    order) where row 0 is the fused mode-0/2 index and row 1 the mode-1
    index. The batch is sorted by the fused index so the big-table HBM
    reads are sequential-ish; `order` maps device position -> original
    row (undo with out[order] = device_out)."""
    i02 = np.asarray(shard[:, 0]).astype(np.int64) * FIELD_DIMS[2] \
        + np.asarray(shard[:, 2])
    i1 = np.asarray(shard[:, 1]).astype(np.int64)
    order = np.arange(i02.shape[0])
    return np.stack([_wrap_idx(i02[order], chunk),
                     _wrap_idx(i1[order], chunk)]), order


def _run(inputs, trace=False, trace_kwargs=None, tdt=None, mdt=None):
    _install_ntff_hook()
    from concourse.bass_utils import run_bass_kernel_spmd

    nc = _build(tdt=tdt, mdt=mdt)
    common = _make_common(inputs, tdt=tdt, mdt=mdt)
    indices = np.asarray(inputs["indices"])
    in_maps, orders = [], []
    for c in range(N_CORES):
        shard = indices[c * BPC:(c + 1) * BPC]
        idxw, order = _make_idxw(shard)
        in_maps.append({**common, "idxw": idxw})
        orders.append(order)

    res = run_bass_kernel_spmd(nc, in_maps, core_ids=list(range(N_CORES)),
                               trace=trace, **(trace_kwargs or {}))
    out = np.empty(B, np.float32)
    for c in range(N_CORES):
        out[c * BPC + orders[c]] = res.results[c]["out"]
    return out, res


def kernel(**inputs):
    out, _ = _run(inputs, trace=False)
    return out
